# Initial kernel scaffold
#
"""Multi-head causal self-attention for TRN2, 8 NeuronCores.

Sharding: core i handles (batch b = i//2, head-group g = i%2); each head-group
is 8 of the 16 heads.  Per core everything is computed in "transposed" space so
no on-device transposes are needed:

  phase 1:  Q^T, K^T [512, T] = W_{q,k}^T @ x^T   (lhsT = W rows, rhs = x^T)
            V [T, 512] = x @ W_v                  (lhsT = x^T chunk, rhs = W_v)
            Q^T stored per-head zero-padded to 128 partitions so attention
            matmuls are full 128x128 shapes (keeps the PE HAM clock-gate warm);
            V staged bf16 as [V_h | 1] per head (+ones tail) for the softmax
            row-sum trick with a full M=128 stationary operand.
  phase 2 (per j-block of 512 query positions):
            per head-pair, per tk-chunk c: S^T(2 heads) = kT_c.T @ qTp into one
            [128,1024] PSUM tile; one ACT exp -> bf16 P^T; causal mask multiply
            on diagonal chunks (DVE); PV accumulate [V_h|1|..].T @ P^T (full
            128x128 bf16) giving O^T rows 0:63 + softmax sums in row 64;
            normalize via approx-reciprocal + K=1 broadcast matmul + DVE mul
            into bf16 Y^T; then the output projection rows for this j-block
            (Y^T.T @ W_proj in bf16) ride behind the ACT-bound attention.
  host sums the two head-group partials per batch and adds b_proj.

QKV/attention-score matmuls in float32r (4x faster than fp32, ~1.5e-4 err);
P/V and the projection in bf16.
"""

import numpy as np
import ml_dtypes
from contextlib import ExitStack

import concourse.bass as bass
import concourse.mybir as mybir
import concourse.tile as tile
from concourse import bacc
from concourse.bass_utils import run_bass_kernel_spmd

B, T, D, H = 4, 2048, 1024, 16
DK = 64            # head dim
HL = 8             # heads per core
DL = HL * DK       # 512 local head dims per core
N_CORES = 8

F32 = mybir.dt.float32
F32R = mybir.dt.float32r
BF16 = mybir.dt.bfloat16
EXP = mybir.ActivationFunctionType.Exp
IDENT = mybir.ActivationFunctionType.Identity

TQ = 512           # tq block size
TKC = 128          # tk chunk size
NQB = T // TQ      # 4
NKC = T // TKC     # 16
NDCH = D // 128    # 8 contraction chunks over D
VSW = HL * 65 + 64  # staged-V width: 8*[V_h|1] + ones tail pad for M=128 lhsT

_CACHE = {}


def _build(causal: bool):
    nc = bacc.Bacc("TRN2", target_bir_lowering=False, debug=False,
                   num_devices=N_CORES)
    xT_d = nc.dram_tensor("xT", [D, T], F32, kind="ExternalInput").ap()
    wqk_d = nc.dram_tensor("wqk", [D, 2 * DL], F32, kind="ExternalInput").ap()
    wv_d = nc.dram_tensor("wv", [D, DL], F32, kind="ExternalInput").ap()
    bqk_d = nc.dram_tensor("bqk", [2 * DL // 128, 128, 1], F32,
                           kind="ExternalInput").ap()
    bv_d = nc.dram_tensor("bv", [1, DL], F32, kind="ExternalInput").ap()
    wp_d = nc.dram_tensor("wproj", [DL, D], F32, kind="ExternalInput").ap()
    masks_d = nc.dram_tensor("masks", [TKC, 4 * TQ], BF16,
                             kind="ExternalInput").ap()
    out_d = nc.dram_tensor("out", [T, D], F32, kind="ExternalOutput").ap()

    with tile.TileContext(nc) as tc, ExitStack() as top:
        persist = top.enter_context(tc.tile_pool(name="persist", bufs=1))

        qTp = [persist.tile([128, T], BF16, tag=f"qTp{h}", name=f"qTp{h}")
               for h in range(HL)]      # per-head, zero-padded other half
        kT = [persist.tile([128, T], BF16, tag=f"kT{i}", name=f"kT{i}")
              for i in range(4)]        # head-pair packed
        vs = [persist.tile([128, VSW], BF16, tag=f"vs{t}", name=f"vs{t}")
              for t in range(NKC)]
        ones_r = persist.tile([1, 128], F32R, tag="ones_r", name="ones_r")
        bqk_sb = [persist.tile([128, 1], F32, tag=f"bqk{m}", name=f"bqk{m}")
                  for m in range(8)]
        for m in range(8):
            nc.gpsimd.dma_start(bqk_sb[m][:], bqk_d[m])
        bv_r = persist.tile([1, DL], F32R, tag="bv_r", name="bv_r")

        # ---------------- phase 1: QKV projections ----------------
        with ExitStack() as ph1:
            wstage = ph1.enter_context(tc.tile_pool(name="wstage", bufs=3))
            wpool = ph1.enter_context(tc.tile_pool(name="wpool", bufs=1))
            xstage = ph1.enter_context(tc.tile_pool(name="xstage", bufs=4))
            xrpool = ph1.enter_context(tc.tile_pool(name="xrpool", bufs=2))
            ps1 = ph1.enter_context(tc.tile_pool(name="ps1", bufs=3, space="PSUM"))

            # small constants + one-time fills
            initp = ph1.enter_context(tc.tile_pool(name="initp", bufs=1))
            ones_f = initp.tile([1, 128], F32, tag="ones_f", name="ones_f")
            nc.vector.memset(ones_f[:], 1.0)
            nc.vector.tensor_copy(ones_r[:], ones_f[:])
            ones8 = initp.tile([128, 64], F32, tag="ones8", name="ones8")
            nc.vector.memset(ones8[:], 1.0)
            bv_f = initp.tile([1, DL], F32, tag="bv_f", name="bv_f")
            nc.gpsimd.dma_start(bv_f[:], bv_d)
            nc.vector.tensor_copy(bv_r[:], bv_f[:])
            zeros = initp.tile([64, TQ], F32, tag="zeros", name="zeros")
            nc.vector.memset(zeros[:], 0.0)
            for h in range(HL):
                pad = slice(64, 128) if h % 2 == 0 else slice(0, 64)
                for jz in range(NQB):
                    nc.vector.tensor_copy(
                        qTp[h][pad, jz * TQ:(jz + 1) * TQ], zeros[:])
            for t in range(NKC):
                for h in range(HL):
                    nc.vector.tensor_copy(
                        vs[t][:, h * 65 + 64:h * 65 + 65], ones8[:, 0:1])
                nc.vector.tensor_copy(vs[t][:, HL * 65:VSW], ones8[:])

            wqk_r, wv_r = [], []
            for d in range(NDCH):
                st = wstage.tile([128, 2 * DL], F32, tag="wqks", name=f"wqks{d}")
                nc.gpsimd.dma_start(st[:], wqk_d[d * 128:(d + 1) * 128, :])
                wr = wpool.tile([128, 2 * DL], BF16, tag=f"wqk{d}", name=f"wqk{d}")
                nc.vector.tensor_copy(wr[:], st[:])
                wqk_r.append(wr)

                stv = wstage.tile([128, DL], F32, tag="wvs", name=f"wvs{d}")
                nc.gpsimd.dma_start(stv[:], wv_d[d * 128:(d + 1) * 128, :])
                wvr = wpool.tile([128, DL], BF16, tag=f"wv{d}", name=f"wv{d}")
                nc.vector.tensor_copy(wvr[:], stv[:])
                wv_r.append(wvr)

            for j in range(NQB):
                jsl = slice(j * TQ, (j + 1) * TQ)
                xr = []
                for d in range(NDCH):
                    st = xstage.tile([128, TQ], F32, tag="xs", name=f"xs{j}_{d}")
                    nc.sync.dma_start(st[:], xT_d[d * 128:(d + 1) * 128, jsl])
                    xrt = xrpool.tile([128, TQ], BF16, tag=f"xr{d}",
                                      name=f"xr{j}_{d}")
                    nc.vector.tensor_copy(xrt[:], st[:])
                    xr.append(xrt)

                for m in range(8):
                    ps = ps1.tile([128, TQ], F32, tag="psqk", name=f"psqk{j}_{m}")
                    for d in range(NDCH):
                        nc.tensor.matmul(
                            ps[:], wqk_r[d][:, m * 128:(m + 1) * 128],
                            xr[d][:], start=(d == 0), stop=(d == NDCH - 1))
                    if m < 4:
                        nc.scalar.activation(
                            qTp[2 * m][0:64, jsl], ps[0:64, :], IDENT,
                            bias=bqk_sb[m][0:64], scale=1.0)
                        nc.scalar.activation(
                            qTp[2 * m + 1][64:128, jsl], ps[64:128, :], IDENT,
                            bias=bqk_sb[m][64:128], scale=1.0)
                    else:
                        nc.scalar.activation(
                            kT[m - 4][:, jsl], ps[:], IDENT,
                            bias=bqk_sb[m][:], scale=1.0)

                for tt in range(4 * j, 4 * j + 4):
                    c = tt % 4
                    ps = ps1.tile([128, DL], F32, tag="psv", name=f"psv{tt}")
                    for d in range(NDCH):
                        nc.tensor.matmul(
                            ps[:], xr[d][:, c * 128:(c + 1) * 128], wv_r[d][:],
                            start=(d == 0), stop=False)
                    nc.tensor.matmul(ps[:], ones_r[:, 0:128], bv_r[:],
                                     start=False, stop=True)
                    for h in range(HL):
                        nc.vector.tensor_copy(vs[tt][:, h * 65:h * 65 + 64],
                                              ps[:, h * 64:(h + 1) * 64])

        # -------- phase 2: attention + projection per j-block --------
        with ExitStack() as ph2:
            maskpool = ph2.enter_context(tc.tile_pool(name="maskpool", bufs=1))
            wpool3 = ph2.enter_context(tc.tile_pool(name="wpool3", bufs=1))
            wstage3 = ph2.enter_context(tc.tile_pool(name="wstage3", bufs=2))
            ypool = ph2.enter_context(tc.tile_pool(name="ypool", bufs=1))
            ps_s = ph2.enter_context(tc.tile_pool(name="ps_s", bufs=2, space="PSUM"))
            ps_o = ph2.enter_context(tc.tile_pool(name="ps_o", bufs=2, space="PSUM"))
            ps_b = ph2.enter_context(tc.tile_pool(name="ps_b", bufs=1, space="PSUM"))
            ps_3 = ph2.enter_context(tc.tile_pool(name="ps_3", bufs=1, space="PSUM"))
            ppool = ph2.enter_context(tc.tile_pool(name="ppool", bufs=6))
            npool = ph2.enter_context(tc.tile_pool(name="npool", bufs=3))
            opool = ph2.enter_context(tc.tile_pool(name="opool", bufs=3))

            maskb = None
            if causal:
                maskb = maskpool.tile([TKC, 4 * TQ], BF16, tag="maskb",
                                      name="maskb")
                nc.gpsimd.dma_start(maskb[:], masks_d)
            yT = [ypool.tile([128, T], BF16, tag=f"yT{i}", name=f"yT{i}")
                  for i in range(4)]
            wp_r = []
            for k in range(4):
                st = wstage3.tile([128, D], F32, tag="wps", name=f"wps{k}")
                nc.gpsimd.dma_start(st[:], wp_d[k * 128:(k + 1) * 128, :])
                wr = wpool3.tile([128, D], BF16, tag=f"wp{k}", name=f"wp{k}")
                nc.vector.tensor_copy(wr[:], st[:])
                wp_r.append(wr)

            def proj_step(t, nb):
                nsl = slice(nb * 512, (nb + 1) * 512)
                ps = ps_3.tile([128, TQ], F32, tag="p3", name=f"ps3_{t}_{nb}")
                for k in range(4):
                    nc.tensor.matmul(
                        ps[:], yT[k][:, t * 128:(t + 1) * 128],
                        wp_r[k][:, nsl], start=(k == 0), stop=(k == 3))
                ot = opool.tile([128, TQ], F32, tag="ot", name=f"ot{t}_{nb}")
                nc.vector.tensor_copy(ot[:], ps[:])
                nc.sync.dma_start(out_d[t * 128:(t + 1) * 128, nsl], ot[:])

            pending = []   # proj steps of block j-1, interleaved into attn(j)
            for j in range(NQB):
                jsl = slice(j * TQ, (j + 1) * TQ)
                cs = list(range(4 * (j + 1))) if causal else list(range(NKC))
                for i in range(4):          # head pair (2i, 2i+1)
                    hA, hB = 2 * i, 2 * i + 1
                    poA = ps_o.tile([128, TQ], F32, tag="po", name=f"poA{j}_{i}")
                    poB = ps_o.tile([128, TQ], F32, tag="po", name=f"poB{j}_{i}")

                    pend = None   # pipeline: PV(c) emitted after QK(c+1)
                    for ci, c in enumerate(cs):
                        csl = slice(c * TKC, (c + 1) * TKC)
                        ss = ps_s.tile([TKC, 2 * TQ], F32, tag="ss",
                                       name=f"ss{j}_{i}_{c}")
                        nc.tensor.matmul(ss[:, 0:TQ], kT[i][:, csl],
                                         qTp[hA][:, jsl], start=True, stop=True)
                        nc.tensor.matmul(ss[:, TQ:2 * TQ], kT[i][:, csl],
                                         qTp[hB][:, jsl], start=True, stop=True)
                        pt = ppool.tile([TKC, 2 * TQ], BF16, tag="pt",
                                        name=f"pt{j}_{i}_{c}")
                        nc.scalar.activation(pt[:], ss[:], EXP, scale=0.125)
                        if causal and c >= 4 * j:
                            s = c - 4 * j
                            msl = slice(s * TQ, (s + 1) * TQ)
                            nc.vector.tensor_mul(pt[:, 0:TQ], pt[:, 0:TQ],
                                                 maskb[:, msl])
                            nc.vector.tensor_mul(pt[:, TQ:2 * TQ],
                                                 pt[:, TQ:2 * TQ], maskb[:, msl])
                        if pend is not None:
                            pc, ppt = pend
                            st = (ci == 1)
                            nc.tensor.matmul(
                                poA[:], vs[pc][:, hA * 65:hA * 65 + 128],
                                ppt[:, 0:TQ], start=st, stop=False)
                            nc.tensor.matmul(
                                poB[:], vs[pc][:, hB * 65:hB * 65 + 128],
                                ppt[:, TQ:2 * TQ], start=st, stop=False)
                        if pending:
                            pending.pop(0)()
                        pend = (c, pt)
                    pc, ppt = pend
                    one = (len(cs) == 1)
                    nc.tensor.matmul(poA[:], vs[pc][:, hA * 65:hA * 65 + 128],
                                     ppt[:, 0:TQ], start=one, stop=True)
                    nc.tensor.matmul(poB[:], vs[pc][:, hB * 65:hB * 65 + 128],
                                     ppt[:, TQ:2 * TQ], start=one, stop=True)

                    # normalize: copy sums+O off PSUM fast, approx-recip in SBUF
                    for h, po in ((hA, poA), (hB, poB)):
                        hp = h % 2
                        sums = npool.tile([1, TQ], F32, tag="sums",
                                          name=f"sm{j}_{h}")
                        nc.vector.tensor_copy(sums[:], po[64:65, :])
                        o_sb = npool.tile([64, TQ], BF16, tag="o_sb",
                                          name=f"ob{j}_{h}")
                        nc.vector.tensor_copy(o_sb[:], po[0:64, :])
                        recip = npool.tile([1, TQ], F32, tag="recip",
                                           name=f"rc{j}_{h}")
                        scr = npool.tile([1, TQ], F32, tag="scr",
                                         name=f"sc{j}_{h}")
                        recip_r = npool.tile([1, TQ], F32R, tag="recip_r",
                                             name=f"rr{j}_{h}")
                        nc.vector.reciprocal_approx_accurate(
                            out=recip[:], in_=sums[:], scratch=scr[:])
                        nc.vector.tensor_copy(recip_r[:], recip[:])
                        pb = ps_b.tile([64, TQ], F32, tag="pb",
                                       name=f"pb{j}_{h}")
                        nc.tensor.matmul(pb[:], ones_r[:, 0:64], recip_r[:],
                                         start=True, stop=True)
                        nc.vector.tensor_mul(
                            yT[i][hp * 64:(hp + 1) * 64, jsl], o_sb[:], pb[:])

                # queue this j-block's projection; emitted inside attn(j+1)
                for t in range(4 * j, 4 * j + 4):
                    for nb in range(2):
                        pending.append(
                            lambda t=t, nb=nb: proj_step(t, nb))
            for fn in pending:   # flush last block's projection
                fn()

    nc.compile()
    return nc


def _get_nc(causal: bool):
    if causal not in _CACHE:
        _CACHE[causal] = _build(causal)
    return _CACHE[causal]


def _host_masks() -> np.ndarray:
    i = np.arange(TKC)[:, None]
    jj = np.arange(TQ)[None, :]
    blocks = [(jj >= i + s * TKC).astype(np.float32) for s in range(4)]
    return np.ascontiguousarray(
        np.concatenate(blocks, axis=1).astype(ml_dtypes.bfloat16))


def _make_in_maps(x, W_qkv, b_qkv, W_proj):
    masks_np = _host_masks()
    in_maps = []
    for core in range(N_CORES):
        b, g = core // 2, core % 2
        qc = slice(g * DL, (g + 1) * DL)
        kc = slice(D + g * DL, D + (g + 1) * DL)
        vc = slice(2 * D + g * DL, 2 * D + (g + 1) * DL)
        in_maps.append({
            "xT": np.ascontiguousarray(x[b].T),
            "wqk": np.ascontiguousarray(
                np.concatenate([W_qkv[:, qc], W_qkv[:, kc]], axis=1)),
            "wv": np.ascontiguousarray(W_qkv[:, vc]),
            "bqk": np.ascontiguousarray(
                np.concatenate([b_qkv[qc], b_qkv[kc]]).reshape(8, 128, 1)),
            "bv": np.ascontiguousarray(b_qkv[vc].reshape(1, DL)),
            "wproj": np.ascontiguousarray(W_proj[g * DL:(g + 1) * DL, :]),
            "masks": masks_np,
        })
    return in_maps


def kernel(x, mask, W_qkv, b_qkv, W_proj, b_proj):
    x = np.asarray(x, dtype=np.float32)
    mask2d = np.asarray(mask, dtype=np.int32).reshape(T, T)
    W_qkv = np.asarray(W_qkv, dtype=np.float32)
    b_qkv = np.asarray(b_qkv, dtype=np.float32)
    W_proj = np.asarray(W_proj, dtype=np.float32)
    b_proj = np.asarray(b_proj, dtype=np.float32)

    if np.array_equal(mask2d, np.tril(np.ones((T, T), dtype=np.int32))):
        causal = True
    elif np.all(mask2d == 1):
        causal = False
    else:
        raise NotImplementedError("only causal (tril) or all-ones masks")

    nc = _get_nc(causal)
    in_maps = _make_in_maps(x, W_qkv, b_qkv, W_proj)
    res = run_bass_kernel_spmd(nc, in_maps, core_ids=list(range(N_CORES)))
    out = np.empty((B, T, D), dtype=np.float32)
    for b in range(B):
        out[b] = (res.results[2 * b]["out"] + res.results[2 * b + 1]["out"]
                  + b_proj[None, :])
    return out



# revision 1
# speedup vs baseline: 1.3642x; 1.3642x over previous
"""Multi-head causal self-attention for TRN2, 8 NeuronCores.

Sharding: core i handles (batch b = i//2, head-group g = i%2); each head-group
is 8 of the 16 heads.  Per core everything is computed in "transposed" space so
no on-device transposes are needed:

  phase 1:  Q^T, K^T [512, T] = W_{q,k}^T @ x^T   (lhsT = W rows, rhs = x^T)
            V [T, 512] = x @ W_v                  (lhsT = x^T chunk, rhs = W_v)
            Q^T stored per-head zero-padded to 128 partitions so attention
            matmuls are full 128x128 shapes (keeps the PE HAM clock-gate warm);
            V staged bf16 as [V_h | 1] per head (+ones tail) for the softmax
            row-sum trick with a full M=128 stationary operand.
  phase 2 (per j-block of 512 query positions):
            per head-pair, per tk-chunk c: S^T(2 heads) = kT_c.T @ qTp into one
            [128,1024] PSUM tile; one ACT exp -> bf16 P^T; causal mask multiply
            on diagonal chunks (DVE); PV accumulate [V_h|1|..].T @ P^T (full
            128x128 bf16) giving O^T rows 0:63 + softmax sums in row 64;
            normalize via approx-reciprocal + K=1 broadcast matmul + DVE mul
            into bf16 Y^T; then the output projection rows for this j-block
            (Y^T.T @ W_proj in bf16) ride behind the ACT-bound attention.
  host sums the two head-group partials per batch and adds b_proj.

QKV/attention-score matmuls in float32r (4x faster than fp32, ~1.5e-4 err);
P/V and the projection in bf16.
"""

import numpy as np
import ml_dtypes
from contextlib import ExitStack

import concourse.bass as bass
import concourse.mybir as mybir
import concourse.tile as tile
from concourse import bacc
from concourse.bass_utils import run_bass_kernel_spmd

B, T, D, H = 4, 2048, 1024, 16
DK = 64            # head dim
HL = 8             # heads per core
DL = HL * DK       # 512 local head dims per core
N_CORES = 8

F32 = mybir.dt.float32
F32R = mybir.dt.float32r
BF16 = mybir.dt.bfloat16
EXP = mybir.ActivationFunctionType.Exp
IDENT = mybir.ActivationFunctionType.Identity

TQ = 512           # tq block size
TKC = 128          # tk chunk size
NQB = T // TQ      # 4
NKC = T // TKC     # 16
NDCH = D // 128    # 8 contraction chunks over D
VSW = HL * 65 + 64  # staged-V width: 8*[V_h|1] + ones tail pad for M=128 lhsT

_CACHE = {}


def _build(causal: bool):
    nc = bacc.Bacc("TRN2", target_bir_lowering=False, debug=False,
                   num_devices=N_CORES)
    xT_d = nc.dram_tensor("xT", [D, T], F32, kind="ExternalInput").ap()
    wqk_d = nc.dram_tensor("wqk", [D, 2 * DL], F32, kind="ExternalInput").ap()
    wv_d = nc.dram_tensor("wv", [D, DL], F32, kind="ExternalInput").ap()
    bqk_d = nc.dram_tensor("bqk", [2 * DL // 128, 128, 1], F32,
                           kind="ExternalInput").ap()
    bv_d = nc.dram_tensor("bv", [1, DL], F32, kind="ExternalInput").ap()
    wp_d = nc.dram_tensor("wproj", [DL, D], F32, kind="ExternalInput").ap()
    masks_d = nc.dram_tensor("masks", [TKC, 4 * TQ], BF16,
                             kind="ExternalInput").ap()
    out_d = nc.dram_tensor("out", [T, D], F32, kind="ExternalOutput").ap()

    with tile.TileContext(nc) as tc, ExitStack() as top:
        persist = top.enter_context(tc.tile_pool(name="persist", bufs=1))

        qTp = [persist.tile([128, T], BF16, tag=f"qTp{h}", name=f"qTp{h}")
               for h in range(HL)]      # per-head, zero-padded other half
        kT = [persist.tile([128, T], BF16, tag=f"kT{i}", name=f"kT{i}")
              for i in range(4)]        # head-pair packed
        vs = [persist.tile([128, VSW], BF16, tag=f"vs{t}", name=f"vs{t}")
              for t in range(NKC)]
        ones_r = persist.tile([1, 128], F32R, tag="ones_r", name="ones_r")
        bqk_sb = [persist.tile([128, 1], F32, tag=f"bqk{m}", name=f"bqk{m}")
                  for m in range(8)]
        for m in range(8):
            nc.gpsimd.dma_start(bqk_sb[m][:], bqk_d[m])
        bv_r = persist.tile([1, DL], F32R, tag="bv_r", name="bv_r")

        # ---------------- phase 1: QKV projections ----------------
        with ExitStack() as ph1:
            wstage = ph1.enter_context(tc.tile_pool(name="wstage", bufs=3))
            wpool = ph1.enter_context(tc.tile_pool(name="wpool", bufs=1))
            xstage = ph1.enter_context(tc.tile_pool(name="xstage", bufs=4))
            xrpool = ph1.enter_context(tc.tile_pool(name="xrpool", bufs=2))
            ps1 = ph1.enter_context(tc.tile_pool(name="ps1", bufs=3, space="PSUM"))

            # small constants + one-time fills
            initp = ph1.enter_context(tc.tile_pool(name="initp", bufs=1))
            ones_f = initp.tile([1, 128], F32, tag="ones_f", name="ones_f")
            nc.vector.memset(ones_f[:], 1.0)
            nc.vector.tensor_copy(ones_r[:], ones_f[:])
            ones8 = initp.tile([128, 64], F32, tag="ones8", name="ones8")
            nc.vector.memset(ones8[:], 1.0)
            bv_f = initp.tile([1, DL], F32, tag="bv_f", name="bv_f")
            nc.gpsimd.dma_start(bv_f[:], bv_d)
            nc.vector.tensor_copy(bv_r[:], bv_f[:])
            zeros = initp.tile([64, TQ], F32, tag="zeros", name="zeros")
            nc.vector.memset(zeros[:], 0.0)
            for h in range(HL):
                pad = slice(64, 128) if h % 2 == 0 else slice(0, 64)
                for jz in range(NQB):
                    nc.vector.tensor_copy(
                        qTp[h][pad, jz * TQ:(jz + 1) * TQ], zeros[:])
            for t in range(NKC):
                for h in range(HL):
                    nc.vector.tensor_copy(
                        vs[t][:, h * 65 + 64:h * 65 + 65], ones8[:, 0:1])
                nc.vector.tensor_copy(vs[t][:, HL * 65:VSW], ones8[:])

            wqk_r, wv_r = [], []
            for d in range(NDCH):
                st = wstage.tile([128, 2 * DL], F32, tag="wqks", name=f"wqks{d}")
                nc.gpsimd.dma_start(st[:], wqk_d[d * 128:(d + 1) * 128, :])
                wr = wpool.tile([128, 2 * DL], BF16, tag=f"wqk{d}", name=f"wqk{d}")
                nc.vector.tensor_copy(wr[:], st[:])
                wqk_r.append(wr)

                stv = wstage.tile([128, DL], F32, tag="wvs", name=f"wvs{d}")
                nc.gpsimd.dma_start(stv[:], wv_d[d * 128:(d + 1) * 128, :])
                wvr = wpool.tile([128, DL], BF16, tag=f"wv{d}", name=f"wv{d}")
                nc.vector.tensor_copy(wvr[:], stv[:])
                wv_r.append(wvr)

            for j in range(NQB):
                jsl = slice(j * TQ, (j + 1) * TQ)
                xr = []
                for d in range(NDCH):
                    st = xstage.tile([128, TQ], F32, tag="xs", name=f"xs{j}_{d}")
                    nc.sync.dma_start(st[:], xT_d[d * 128:(d + 1) * 128, jsl])
                    xrt = xrpool.tile([128, TQ], BF16, tag=f"xr{d}",
                                      name=f"xr{j}_{d}")
                    nc.vector.tensor_copy(xrt[:], st[:])
                    xr.append(xrt)

                for m in range(8):
                    ps = ps1.tile([128, TQ], F32, tag="psqk", name=f"psqk{j}_{m}")
                    for d in range(NDCH):
                        nc.tensor.matmul(
                            ps[:], wqk_r[d][:, m * 128:(m + 1) * 128],
                            xr[d][:], start=(d == 0), stop=(d == NDCH - 1))
                    if m < 4:
                        nc.scalar.activation(
                            qTp[2 * m][0:64, jsl], ps[0:64, :], IDENT,
                            bias=bqk_sb[m][0:64], scale=1.0)
                        nc.scalar.activation(
                            qTp[2 * m + 1][64:128, jsl], ps[64:128, :], IDENT,
                            bias=bqk_sb[m][64:128], scale=1.0)
                    else:
                        nc.scalar.activation(
                            kT[m - 4][:, jsl], ps[:], IDENT,
                            bias=bqk_sb[m][:], scale=1.0)

                for tt in range(4 * j, 4 * j + 4):
                    c = tt % 4
                    ps = ps1.tile([128, DL], F32, tag="psv", name=f"psv{tt}")
                    for d in range(NDCH):
                        nc.tensor.matmul(
                            ps[:], xr[d][:, c * 128:(c + 1) * 128], wv_r[d][:],
                            start=(d == 0), stop=False)
                    nc.tensor.matmul(ps[:], ones_r[:, 0:128], bv_r[:],
                                     start=False, stop=True)
                    for h in range(HL):
                        nc.vector.tensor_copy(vs[tt][:, h * 65:h * 65 + 64],
                                              ps[:, h * 64:(h + 1) * 64])

        # -------- phase 2: attention + projection per j-block --------
        with ExitStack() as ph2:
            maskpool = ph2.enter_context(tc.tile_pool(name="maskpool", bufs=1))
            wpool3 = ph2.enter_context(tc.tile_pool(name="wpool3", bufs=1))
            wstage3 = ph2.enter_context(tc.tile_pool(name="wstage3", bufs=2))
            ypool = ph2.enter_context(tc.tile_pool(name="ypool", bufs=1))
            ps_s = ph2.enter_context(tc.tile_pool(name="ps_s", bufs=2, space="PSUM"))
            ps_o = ph2.enter_context(tc.tile_pool(name="ps_o", bufs=2, space="PSUM"))
            ps_b = ph2.enter_context(tc.tile_pool(name="ps_b", bufs=1, space="PSUM"))
            ps_3 = ph2.enter_context(tc.tile_pool(name="ps_3", bufs=1, space="PSUM"))
            ppool = ph2.enter_context(tc.tile_pool(name="ppool", bufs=6))
            npool = ph2.enter_context(tc.tile_pool(name="npool", bufs=3))
            opool = ph2.enter_context(tc.tile_pool(name="opool", bufs=3))

            maskb = None
            if causal:
                maskb = maskpool.tile([TKC, 4 * TQ], BF16, tag="maskb",
                                      name="maskb")
                nc.gpsimd.dma_start(maskb[:], masks_d)
            yT = [ypool.tile([128, T], BF16, tag=f"yT{i}", name=f"yT{i}")
                  for i in range(4)]
            wp_r = []
            for k in range(4):
                st = wstage3.tile([128, D], F32, tag="wps", name=f"wps{k}")
                nc.gpsimd.dma_start(st[:], wp_d[k * 128:(k + 1) * 128, :])
                wr = wpool3.tile([128, D], BF16, tag=f"wp{k}", name=f"wp{k}")
                nc.vector.tensor_copy(wr[:], st[:])
                wp_r.append(wr)

            def proj_step(t, nb):
                nsl = slice(nb * 512, (nb + 1) * 512)
                ps = ps_3.tile([128, TQ], F32, tag="p3", name=f"ps3_{t}_{nb}")
                for k in range(4):
                    nc.tensor.matmul(
                        ps[:], yT[k][:, t * 128:(t + 1) * 128],
                        wp_r[k][:, nsl], start=(k == 0), stop=(k == 3))
                ot = opool.tile([128, TQ], F32, tag="ot", name=f"ot{t}_{nb}")
                nc.vector.tensor_copy(ot[:], ps[:])
                nc.sync.dma_start(out_d[t * 128:(t + 1) * 128, nsl], ot[:])

            pending = []   # proj steps of block j-1, interleaved into attn(j)
            for j in range(NQB):
                jsl = slice(j * TQ, (j + 1) * TQ)
                cs = list(range(4 * (j + 1))) if causal else list(range(NKC))
                for i in range(4):          # head pair (2i, 2i+1)
                    hA, hB = 2 * i, 2 * i + 1
                    poA = ps_o.tile([128, TQ], F32, tag="po", name=f"poA{j}_{i}")
                    poB = ps_o.tile([128, TQ], F32, tag="po", name=f"poB{j}_{i}")

                    pend = None   # pipeline: PV(c) emitted after QK(c+1)
                    for ci, c in enumerate(cs):
                        csl = slice(c * TKC, (c + 1) * TKC)
                        ss = ps_s.tile([TKC, 2 * TQ], F32, tag="ss",
                                       name=f"ss{j}_{i}_{c}")
                        nc.tensor.matmul(ss[:, 0:TQ], kT[i][:, csl],
                                         qTp[hA][:, jsl], start=True, stop=True)
                        nc.tensor.matmul(ss[:, TQ:2 * TQ], kT[i][:, csl],
                                         qTp[hB][:, jsl], start=True, stop=True)
                        pt = ppool.tile([TKC, 2 * TQ], BF16, tag="pt",
                                        name=f"pt{j}_{i}_{c}")
                        nc.scalar.activation(pt[:], ss[:], EXP, scale=0.125)
                        if causal and c >= 4 * j:
                            s = c - 4 * j
                            msl = slice(s * TQ, (s + 1) * TQ)
                            nc.vector.tensor_mul(pt[:, 0:TQ], pt[:, 0:TQ],
                                                 maskb[:, msl])
                            nc.vector.tensor_mul(pt[:, TQ:2 * TQ],
                                                 pt[:, TQ:2 * TQ], maskb[:, msl])
                        if pend is not None:
                            pc, ppt = pend
                            st = (ci == 1)
                            nc.tensor.matmul(
                                poA[:], vs[pc][:, hA * 65:hA * 65 + 128],
                                ppt[:, 0:TQ], start=st, stop=False)
                            nc.tensor.matmul(
                                poB[:], vs[pc][:, hB * 65:hB * 65 + 128],
                                ppt[:, TQ:2 * TQ], start=st, stop=False)
                        if pending:
                            pending.pop(0)()
                        pend = (c, pt)
                    pc, ppt = pend
                    one = (len(cs) == 1)
                    nc.tensor.matmul(poA[:], vs[pc][:, hA * 65:hA * 65 + 128],
                                     ppt[:, 0:TQ], start=one, stop=True)
                    nc.tensor.matmul(poB[:], vs[pc][:, hB * 65:hB * 65 + 128],
                                     ppt[:, TQ:2 * TQ], start=one, stop=True)

                    # normalize: copy sums+O off PSUM fast, approx-recip in SBUF
                    for h, po in ((hA, poA), (hB, poB)):
                        hp = h % 2
                        sums = npool.tile([1, TQ], F32, tag="sums",
                                          name=f"sm{j}_{h}")
                        nc.vector.tensor_copy(sums[:], po[64:65, :])
                        o_sb = npool.tile([64, TQ], BF16, tag="o_sb",
                                          name=f"ob{j}_{h}")
                        nc.vector.tensor_copy(o_sb[:], po[0:64, :])
                        recip = npool.tile([1, TQ], F32, tag="recip",
                                           name=f"rc{j}_{h}")
                        scr = npool.tile([1, TQ], F32, tag="scr",
                                         name=f"sc{j}_{h}")
                        recip_r = npool.tile([1, TQ], F32R, tag="recip_r",
                                             name=f"rr{j}_{h}")
                        nc.vector.reciprocal_approx_accurate(
                            out=recip[:], in_=sums[:], scratch=scr[:])
                        nc.vector.tensor_copy(recip_r[:], recip[:])
                        pb = ps_b.tile([64, TQ], F32, tag="pb",
                                       name=f"pb{j}_{h}")
                        nc.tensor.matmul(pb[:], ones_r[:, 0:64], recip_r[:],
                                         start=True, stop=True)
                        nc.vector.tensor_mul(
                            yT[i][hp * 64:(hp + 1) * 64, jsl], o_sb[:], pb[:])

                # queue this j-block's projection; emitted inside attn(j+1)
                for t in range(4 * j, 4 * j + 4):
                    for nb in range(2):
                        pending.append(
                            lambda t=t, nb=nb: proj_step(t, nb))
            for fn in pending:   # flush last block's projection
                fn()

    nc.compile()
    return nc


def _get_nc(causal: bool):
    if causal not in _CACHE:
        _CACHE[causal] = _build(causal)
    return _CACHE[causal]


def _host_masks() -> np.ndarray:
    i = np.arange(TKC)[:, None]
    jj = np.arange(TQ)[None, :]
    blocks = [(jj >= i + s * TKC).astype(np.float32) for s in range(4)]
    return np.ascontiguousarray(
        np.concatenate(blocks, axis=1).astype(ml_dtypes.bfloat16))


def _make_in_maps(x, W_qkv, b_qkv, W_proj):
    masks_np = _host_masks()
    in_maps = []
    for core in range(N_CORES):
        b, g = core // 2, core % 2
        qc = slice(g * DL, (g + 1) * DL)
        kc = slice(D + g * DL, D + (g + 1) * DL)
        vc = slice(2 * D + g * DL, 2 * D + (g + 1) * DL)
        in_maps.append({
            "xT": np.ascontiguousarray(x[b].T),
            "wqk": np.ascontiguousarray(
                np.concatenate([W_qkv[:, qc], W_qkv[:, kc]], axis=1)),
            "wv": np.ascontiguousarray(W_qkv[:, vc]),
            "bqk": np.ascontiguousarray(
                np.concatenate([b_qkv[qc], b_qkv[kc]]).reshape(8, 128, 1)),
            "bv": np.ascontiguousarray(b_qkv[vc].reshape(1, DL)),
            "wproj": np.ascontiguousarray(W_proj[g * DL:(g + 1) * DL, :]),
            "masks": masks_np,
        })
    return in_maps


def kernel(x, mask, W_qkv, b_qkv, W_proj, b_proj):
    x = np.asarray(x, dtype=np.float32)
    mask2d = np.asarray(mask, dtype=np.int32).reshape(T, T)
    W_qkv = np.asarray(W_qkv, dtype=np.float32)
    b_qkv = np.asarray(b_qkv, dtype=np.float32)
    W_proj = np.asarray(W_proj, dtype=np.float32)
    b_proj = np.asarray(b_proj, dtype=np.float32)

    if np.array_equal(mask2d, np.tril(np.ones((T, T), dtype=np.int32))):
        causal = True
    elif np.all(mask2d == 1):
        causal = False
    else:
        raise NotImplementedError("only causal (tril) or all-ones masks")

    nc = _get_nc(causal)
    in_maps = _make_in_maps(x, W_qkv, b_qkv, W_proj)
    res = run_bass_kernel_spmd(nc, in_maps, core_ids=list(range(N_CORES)))
    out = np.empty((B, T, D), dtype=np.float32)
    for b in range(B):
        out[b] = (res.results[2 * b]["out"] + res.results[2 * b + 1]["out"]
                  + b_proj[None, :])
    return out

